# revision 1
# baseline (speedup 1.0000x reference)
"""CenterLoss forward on 8 Trainium2 NeuronCores.

Reference semantics:
    distmat[b, c] = ||x_b||^2 + ||center_c||^2 - 2 <x_b, center_c>
    loss = sum(clip(distmat * onehot(labels), 1e-12, 1e12)) / B

The masked matrix is zero everywhere except (b, labels[b]), and clip() lifts
each of the B*(C-1) zeros to exactly 1e-12.  So:

    loss = ( sum_b clip(||x_b - centers[labels[b]]||^2, 1e-12, 1e12)
             + B*(C-1)*1e-12 ) / B

which needs only a row gather + per-row squared distance, not the full
(B, C) distance matrix (42 GFLOP -> ~4 MFLOP).

Device kernel (raw Bass, single basic block, SPMD data-parallel over batch):
  - centers are baked into the NEFF as a Const tensor (they are module
    *state* in the reference nn.Module); the runtime DMAs them to HBM at
    model-load time, so per-execution I/O is just the x shard + labels.
  - per core: 512 rows = 4 chunks of 128 partitions
      gpsimd:  label load, then 4 indirect-DMA row gathers
               centers[labels] -> SBUF (alternating two SWDGE queues),
               plus a tiny trailing dummy DMA that flushes the last
               gather's completion receipt through the lane promptly
      sync (HWDGE): the 4 x-chunk loads, one sem per DMA
      vector (DVE): subtract, fused square+row-reduce
               (scalar_tensor_tensor accum_out), clip
  - sync rules learned the hard way (sim race detector + hardware):
      * SWDGE/HWDGE descriptors complete out of order across rings; a
        semaphore value only proves HOW MANY of its increments landed,
        so every DMA whose completion matters gets its own semaphore
        (or a dedicated per-chunk one).
      * SWDGE sems may not be shared with HWDGE DMAs (must start at 0).
      * same-engine RAW on DVE needs an explicit sem edge.
  - per-core output: [128, 4] clipped per-row distances; host sums in
    f64, adds the analytic clip floor B*(C-1)*1e-12, divides by B.
"""

import hashlib
from contextlib import ExitStack

import numpy as np

import concourse.bass as bass
from concourse import mybir
from concourse.bass_utils import run_bass_kernel_spmd

B = 4096
D = 512
C = 10000
NCORES = 8
BL = B // NCORES          # 512 rows per core
P = 128                   # partitions
NT = BL // P              # 4 chunks per core

F32 = mybir.dt.float32
I32 = mybir.dt.int32

_CACHE = {}


def legalize_waits(nc, max_waits=1):
    """The walrus build in this container accepts at most one embedded
    sem-wait per TPB instruction ("Too many sync wait commands" otherwise).
    Split any excess into standalone single-wait InstEventSemaphore no-ops
    immediately before the instruction on the same engine — engine program
    order then enforces the identical synchronization."""
    n_split = 0
    for f in nc.m.functions:
        for b in f.blocks:
            insts = list(b.instructions)
            out = []
            for inst in insts:
                si = inst.sync_info
                waits = list(si.on_wait) if (si is not None and si.on_wait) else []
                if len(waits) > max_waits:
                    keep = waits[-max_waits:]
                    spill = waits[:-max_waits]
                    for k, w in enumerate(spill):
                        out.append(
                            mybir.InstEventSemaphore(
                                name=f"{inst.name}-lw{k}",
                                engine=inst.engine,
                                sync_info=mybir.SyncInfo(on_wait=[w], on_update=[]),
                            )
                        )
                        n_split += 1
                    inst.sync_info = mybir.SyncInfo(
                        on_wait=keep, on_update=list(si.on_update or [])
                    )
                out.append(inst)
            b.instructions = out
    return n_split


def build_nc(centers_np):
    nc = bass.Bass(num_swdge_queues=2)

    x = nc.dram_tensor("x", [BL, D], F32, kind="ExternalInput")
    # labels pre-arranged on host: [p, t] = original label[t*128 + p]
    labels = nc.dram_tensor("labels", [P, NT], I32, kind="ExternalInput")
    out = nc.dram_tensor("out", [P, NT], F32, kind="ExternalOutput")
    centers = nc.inline_tensor(
        np.ascontiguousarray(centers_np, dtype=np.float32), name="centers"
    )

    es = ExitStack()
    idx_sb = es.enter_context(nc.sbuf_tensor("idx_sb", [P, NT], I32))
    x_sb = es.enter_context(nc.sbuf_tensor("x_sb", [P, NT * D], F32))
    c_sb = es.enter_context(nc.sbuf_tensor("c_sb", [P, NT * D], F32))
    df_sb = es.enter_context(nc.sbuf_tensor("df_sb", [P, NT * D], F32))
    sq_sb = es.enter_context(nc.sbuf_tensor("sq_sb", [P, NT * D], F32))
    dcols = es.enter_context(nc.sbuf_tensor("dcols", [P, NT], F32))
    dclip = es.enter_context(nc.sbuf_tensor("dclip", [P, NT], F32))
    scr_sb = es.enter_context(nc.sbuf_tensor("scr_sb", [P, NT], I32))
    scr2_sb = es.enter_context(nc.sbuf_tensor("scr2_sb", [P, NT], I32))
    idx_sem = es.enter_context(nc.semaphore("idx_sem"))
    c_sems = [es.enter_context(nc.semaphore(f"c_sem{t}")) for t in range(NT)]
    xc_sems = [es.enter_context(nc.semaphore(f"xc_sem{t}")) for t in range(NT)]
    v_sem = es.enter_context(nc.semaphore("v_sem"))
    o_sem = es.enter_context(nc.semaphore("o_sem"))
    dve_sem = es.enter_context(nc.semaphore("dve_sem"))
    f_sem = es.enter_context(nc.semaphore("f_sem"))

    # ---- gpsimd: labels, then the gathers ----
    nc.gpsimd.dma_start(out=idx_sb[:, :], in_=labels[:, :]).then_inc(idx_sem, 16)
    # dummy DMA right behind the label load: the lane processes it next,
    # which pushes the label DMA's completion receipt through promptly
    # (otherwise idx_sem fires ~2 us late while the lane idles)
    nc.gpsimd.dma_start(out=scr2_sb[:, :], in_=labels[:, :]).then_inc(f_sem, 16)
    # ---- sync/HWDGE: x chunks in parallel with the above ----
    for t in range(NT):
        nc.sync.dma_start(
            out=x_sb[:, t * D:(t + 1) * D], in_=x[t * P:(t + 1) * P, :]
        ).then_inc(xc_sems[t], 16)
    nc.gpsimd.wait_ge(idx_sem, 16)  # indices resident before gathers
    gather_insts = []
    for t in range(NT):
        gi = nc.gpsimd.indirect_dma_start(
            out=c_sb[:, t * D:(t + 1) * D],
            out_offset=None,
            in_=centers[:],
            in_offset=bass.IndirectOffsetOnAxis(ap=idx_sb[:, t:t + 1], axis=0),
        ).then_inc(c_sems[t], 16)
        gather_insts.append(gi)
    # trailing dummy SWDGE DMA: flushes the last gather's completion receipt
    nc.gpsimd.dma_start(out=scr_sb[:, :], in_=labels[:, :]).then_inc(f_sem, 16)

    # ---- vector: per-chunk subtract + fused square/row-reduce ----
    n_dve = 0
    for t in range(NT):
        cs = slice(t * D, (t + 1) * D)
        nc.vector.wait_ge(xc_sems[t], 16)
        nc.vector.wait_ge(c_sems[t], 16)
        nc.vector.tensor_tensor(
            out=df_sb[:, cs],
            in0=x_sb[:, cs],
            in1=c_sb[:, cs],
            op=mybir.AluOpType.subtract,
        ).then_inc(dve_sem, 1)
        n_dve += 1
        nc.vector.wait_ge(dve_sem, n_dve)
        nc.vector.scalar_tensor_tensor(
            out=sq_sb[:, cs],
            in0=df_sb[:, cs],
            scalar=1.0,
            in1=df_sb[:, cs],
            op0=mybir.AluOpType.mult,
            op1=mybir.AluOpType.mult,
            accum_out=dcols[:, t:t + 1],
        ).then_inc(dve_sem, 1)
        n_dve += 1
    nc.vector.wait_ge(dve_sem, n_dve)
    nc.vector.tensor_scalar(
        out=dclip[:, :],
        in0=dcols[:, :],
        scalar1=1e-12,
        scalar2=1e12,
        op0=mybir.AluOpType.max,
        op1=mybir.AluOpType.min,
    ).then_inc(v_sem, 1)

    # ---- result out; runtime drains rings before reading outputs ----
    nc.gpsimd.wait_ge(v_sem, 1)
    nc.gpsimd.dma_start(out=out[:, :], in_=dclip[:, :]).then_inc(o_sem, 16)

    # alternate gathers across the two SWDGE queues
    for t, gi in enumerate(gather_insts):
        if t % 2 == 1:
            gi.ins.queue = "qPoolDynamic1"

    # NOTE: the ExitStack is intentionally NOT closed — closing would free
    # the semaphores and emit an expensive end-of-program drain + barrier;
    # Bass already clears the whole sem range in its preamble, so repeated
    # executions stay safe without it.
    legalize_waits(nc)
    return nc


def _get_nc(centers_np):
    arr = np.ascontiguousarray(centers_np, np.float32)
    key = hashlib.md5(arr.tobytes()).hexdigest()
    if _CACHE.get("key") != key:
        _CACHE["nc"] = build_nc(arr)
        _CACHE["key"] = key
    return _CACHE["nc"]


def make_in_maps(x, labels, centers=None):
    x = np.ascontiguousarray(np.asarray(x, dtype=np.float32))
    # [p, t] = label[t*128 + p] within each core's 512-row shard
    labels_i32 = np.ascontiguousarray(
        np.asarray(labels).astype(np.int32).reshape(NCORES, NT, P).transpose(0, 2, 1)
    )
    xs = x.reshape(NCORES, BL, D)
    return [{"x": xs[i], "labels": labels_i32[i]} for i in range(NCORES)]


def finalize(results):
    total = 0.0
    for r in results:
        total += float(np.asarray(r["out"], dtype=np.float64).sum())
    loss = (total + B * (C - 1) * 1e-12) / B
    return np.array(loss, dtype=np.float32)


def kernel(x, labels, centers):
    nc = _get_nc(centers)
    in_maps = make_in_maps(x, labels)
    res = run_bass_kernel_spmd(nc, in_maps, core_ids=list(range(NCORES)))
    return finalize(res.results)



# revision 5
# speedup vs baseline: 1.1719x; 1.1719x over previous
"""CenterLoss forward on 8 Trainium2 NeuronCores.

Reference semantics:
    distmat[b, c] = ||x_b||^2 + ||center_c||^2 - 2 <x_b, center_c>
    loss = sum(clip(distmat * onehot(labels), 1e-12, 1e12)) / B

The masked matrix is zero everywhere except (b, labels[b]), and clip() lifts
each of the B*(C-1) zeros to exactly 1e-12.  So:

    loss = ( sum_b clip(||x_b - centers[labels[b]]||^2, 1e-12, 1e12)
             + B*(C-1)*1e-12 ) / B

which needs only a row gather + per-row squared distance, not the full
(B, C) distance matrix.

v2 device kernel (raw Bass, SPMD data-parallel over batch), latency-optimized:
  - centers are baked into the NEFF as a Const bf16 table of 513 columns:
    cols 0..511 = centers, col 512 = ||center||^2 (csq) computed on host in
    f32.  One gathered row then carries everything chunk t needs.
  - x is fed as bf16 (host converts); 2e-2 rel tolerance dwarfs the ~1e-4
    error this costs (all DVE accumulation stays f32).
  - per core: 512 rows = 4 chunks of 128 partitions
      sync (HWDGE):  label load FIRST (hoisted before the ctor barrier so it
                     issues at window start), then the 4 x-chunk loads
      gpsimd:        4 indirect-DMA row gathers centers_aug[labels] -> SBUF
                     (alternating two SWDGE queues), then one trailing dummy
                     DMA per queue to flush the last gathers' completion
                     receipts promptly
      vector (DVE):  per chunk, OFF the critical path: fused square+row-sum
                     sxq_t = sum(x_t*x_t) (STT, needs only x) and a bf16->f32
                     cast of the gathered csq column; ON the critical path
                     after gather t lands: ONE fused STT row-sum
                     xc_t = sum((-2 * x_t) * c_t), then two tiny [128,4] adds
                       outv = (sxq + csqf) + xc
                     (tensor_tensor_reduce would fold the adds in, but this
                     walrus build cannot codegen it - "ISA wrong length")
      scalar (ACT):  result DMA out on the idle Activation HWDGE queue
  - per-core output: [128, 4] per-row squared distances; host clips
    (identical semantics: clip acts elementwise on the masked entries),
    sums in f64, adds the analytic clip floor B*(C-1)*1e-12, divides by B.
  - sync rules (from v1, sim race detector + hardware):
      * every DMA whose completion matters gets its own semaphore
      * same-engine RAW on DVE needs an explicit sem edge; dve_sem counts
        completed DVE ops (in-order completion => count k proves ops 1..k,
        including their accum_out drains)
      * SWDGE sems may not be shared with HWDGE DMAs
"""

import hashlib
from contextlib import ExitStack

import ml_dtypes
import numpy as np

import concourse.bass as bass
from concourse import mybir
from concourse.bass_utils import run_bass_kernel_spmd

B = 4096
D = 512
C = 10000
NCORES = 8
BL = B // NCORES          # 512 rows per core
P = 128                   # partitions
NT = BL // P              # 4 chunks per core
DA = D + 1                # augmented row: centers row + csq

F32 = mybir.dt.float32
BF16 = mybir.dt.bfloat16
I32 = mybir.dt.int32

_CACHE = {}


def legalize_waits(nc, max_waits=1):
    """The walrus build in this container accepts at most one embedded
    sem-wait per TPB instruction ("Too many sync wait commands" otherwise).
    Split any excess into standalone single-wait InstEventSemaphore no-ops
    immediately before the instruction on the same engine — engine program
    order then enforces the identical synchronization."""
    n_split = 0
    for f in nc.m.functions:
        for b in f.blocks:
            insts = list(b.instructions)
            out = []
            for inst in insts:
                si = inst.sync_info
                waits = list(si.on_wait) if (si is not None and si.on_wait) else []
                if len(waits) > max_waits:
                    keep = waits[-max_waits:]
                    spill = waits[:-max_waits]
                    for k, w in enumerate(spill):
                        out.append(
                            mybir.InstEventSemaphore(
                                name=f"{inst.name}-lw{k}",
                                engine=inst.engine,
                                sync_info=mybir.SyncInfo(on_wait=[w], on_update=[]),
                            )
                        )
                        n_split += 1
                    inst.sync_info = mybir.SyncInfo(
                        on_wait=keep, on_update=list(si.on_update or [])
                    )
                out.append(inst)
            b.instructions = out
    return n_split


def hoist_before_preamble(nc, inst_names):
    """Move the named instructions to the front of the main block, before the
    Bass-ctor const-AP memsets and all-engine barrier.  Only legal for
    instructions whose engine-side dependencies are register-free DMAs that
    touch no const APs: the owning engine then issues them ahead of its
    barrier arrival, overlapping the DMA latency with the preamble."""
    blk = nc.m.functions[0].blocks[0]
    insts = list(blk.instructions)
    moved = [i for i in insts if i.name in inst_names]
    rest = [i for i in insts if i.name not in inst_names]
    # keep the dummycall first (walrus uses it for the dge table)
    assert rest and type(rest[0]).__name__ == "InstCall"
    blk.instructions = [rest[0]] + moved + rest[1:]
    return len(moved)


def build_nc(centers_np):
    nc = bass.Bass(num_swdge_queues=2)

    x = nc.dram_tensor("x", [BL, D], BF16, kind="ExternalInput")
    # labels pre-arranged on host: [p, t] = original label[t*128 + p]
    labels = nc.dram_tensor("labels", [P, NT], I32, kind="ExternalInput")
    out = nc.dram_tensor("out", [P, NT], F32, kind="ExternalOutput")

    cen = np.ascontiguousarray(centers_np, dtype=np.float32)
    csq = np.sum(cen * cen, axis=1, dtype=np.float32)
    cen_aug = np.concatenate([cen, csq[:, None]], axis=1).astype(ml_dtypes.bfloat16)
    centers = nc.inline_tensor(np.ascontiguousarray(cen_aug), name="centers")

    es = ExitStack()
    idx_sb = es.enter_context(nc.sbuf_tensor("idx_sb", [P, NT], I32))
    x_sb = es.enter_context(nc.sbuf_tensor("x_sb", [P, NT * D], BF16))
    c_sb = es.enter_context(nc.sbuf_tensor("c_sb", [P, NT * DA], BF16))
    junkx = es.enter_context(nc.sbuf_tensor("junkx", [P, NT * D], BF16))
    junkc = es.enter_context(nc.sbuf_tensor("junkc", [P, NT * D], BF16))
    sxq = es.enter_context(nc.sbuf_tensor("sxq", [P, NT], F32))
    csqf = es.enter_context(nc.sbuf_tensor("csqf", [P, NT], F32))
    xcs = es.enter_context(nc.sbuf_tensor("xcs", [P, NT], F32))
    init = es.enter_context(nc.sbuf_tensor("init", [P, NT], F32))
    outv = es.enter_context(nc.sbuf_tensor("outv", [P, NT], F32))
    scr_sb = es.enter_context(nc.sbuf_tensor("scr_sb", [P, NT], I32))
    scr2_sb = es.enter_context(nc.sbuf_tensor("scr2_sb", [P, NT], I32))
    idx_sem = es.enter_context(nc.semaphore("idx_sem"))
    c_sems = [es.enter_context(nc.semaphore(f"c_sem{t}")) for t in range(NT)]
    xc_sems = [es.enter_context(nc.semaphore(f"xc_sem{t}")) for t in range(NT)]
    v_sem = es.enter_context(nc.semaphore("v_sem"))
    o_sem = es.enter_context(nc.semaphore("o_sem"))
    dve_sem = es.enter_context(nc.semaphore("dve_sem"))
    f_sem = es.enter_context(nc.semaphore("f_sem"))

    # ---- sync/HWDGE: labels first (hoisted below), then x chunks ----
    lab_dma = nc.sync.dma_start(out=idx_sb[:, :], in_=labels[:, :])
    lab_dma.then_inc(idx_sem, 16)
    for t in range(NT):
        nc.sync.dma_start(
            out=x_sb[:, t * D:(t + 1) * D], in_=x[t * P:(t + 1) * P, :]
        ).then_inc(xc_sems[t], 16)

    # ---- gpsimd: the gathers (513-wide augmented rows) ----
    nc.gpsimd.wait_ge(idx_sem, 16)  # indices resident before gathers
    gather_insts = []
    for t in range(NT):
        gi = nc.gpsimd.indirect_dma_start(
            out=c_sb[:, t * DA:(t + 1) * DA],
            out_offset=None,
            in_=centers[:],
            in_offset=bass.IndirectOffsetOnAxis(ap=idx_sb[:, t:t + 1], axis=0),
        ).then_inc(c_sems[t], 16)
        gather_insts.append(gi)
    # trailing dummy SWDGE DMA per queue: flushes the last gathers' receipts
    fl0 = nc.gpsimd.dma_start(out=scr_sb[:, :], in_=labels[:, :])
    fl0.then_inc(f_sem, 16)
    fl1 = nc.gpsimd.dma_start(out=scr2_sb[:, :], in_=labels[:, :])
    fl1.then_inc(f_sem, 16)

    # alternate gathers across the two SWDGE queues; flushes one per queue
    for t, gi in enumerate(gather_insts):
        if t % 2 == 1:
            gi.ins.queue = "qPoolDynamic1"
    fl1.ins.queue = "qPoolDynamic1"

    # ---- vector: fused STT row-sums, f32 accumulators ----
    ndve = 0

    def csq_col(t):
        return c_sb[:, t * DA + D: t * DA + D + 1]

    def xs(t):
        return slice(t * D, (t + 1) * D)

    def cs(t):
        return slice(t * DA, t * DA + D)

    # off critical path: sxq_t = sum(x_t * x_t)  (needs only the x chunk)
    for t in range(NT):
        nc.vector.wait_ge(xc_sems[t], 16)
        nc.vector.scalar_tensor_tensor(
            out=junkx[:, xs(t)],
            in0=x_sb[:, xs(t)],
            scalar=1.0,
            in1=x_sb[:, xs(t)],
            op0=mybir.AluOpType.mult,
            op1=mybir.AluOpType.mult,
            accum_out=sxq[:, t:t + 1],
        ).then_inc(dve_sem, 1)
        ndve += 1  # op t+1

    # per chunk: cast csq_t to f32 (tiny, shadowed for t<3), then the one
    # critical op: xc_t = sum((-2 x_t) * c_t)
    for t in range(NT):
        nc.vector.wait_ge(c_sems[t], 16)
        nc.vector.tensor_scalar(
            out=csqf[:, t:t + 1],
            in0=csq_col(t),
            scalar1=1.0,
            scalar2=None,
            op0=mybir.AluOpType.mult,
        ).then_inc(dve_sem, 1)
        ndve += 1
        nc.vector.scalar_tensor_tensor(
            out=junkc[:, xs(t)],
            in0=x_sb[:, xs(t)],
            scalar=-2.0,
            in1=c_sb[:, cs(t)],
            op0=mybir.AluOpType.mult,
            op1=mybir.AluOpType.mult,
            accum_out=xcs[:, t:t + 1],
        ).then_inc(dve_sem, 1)
        ndve += 1

    # final combine: outv = (sxq + csqf) + xcs   (two tiny [128, NT] adds)
    nc.vector.wait_ge(dve_sem, ndve)  # all accums drained (in-order DVE)
    nc.vector.tensor_tensor(
        out=init[:, :],
        in0=sxq[:, :],
        in1=csqf[:, :],
        op=mybir.AluOpType.add,
    ).then_inc(dve_sem, 1)
    ndve += 1
    nc.vector.wait_ge(dve_sem, ndve)
    nc.vector.tensor_tensor(
        out=outv[:, :],
        in0=init[:, :],
        in1=xcs[:, :],
        op=mybir.AluOpType.add,
    ).then_inc(v_sem, 1)

    # ---- result out on the idle Activation HWDGE queue ----
    nc.scalar.wait_ge(v_sem, 1)
    nc.scalar.dma_start(out=out[:, :], in_=outv[:, :]).then_inc(o_sem, 16)

    # NOTE: the ExitStack is intentionally NOT closed — closing would free
    # the semaphores and emit an expensive end-of-program drain + barrier.
    hoist_before_preamble(nc, {lab_dma.ins.name})
    legalize_waits(nc)
    return nc


def _get_nc(centers_np):
    arr = np.ascontiguousarray(centers_np, np.float32)
    key = hashlib.md5(arr.tobytes()).hexdigest()
    if _CACHE.get("key") != key:
        _CACHE["nc"] = build_nc(arr)
        _CACHE["key"] = key
    return _CACHE["nc"]


def make_in_maps(x, labels, centers=None):
    x = np.ascontiguousarray(np.asarray(x, dtype=np.float32)).astype(ml_dtypes.bfloat16)
    # [p, t] = label[t*128 + p] within each core's 512-row shard
    labels_i32 = np.ascontiguousarray(
        np.asarray(labels).astype(np.int32).reshape(NCORES, NT, P).transpose(0, 2, 1)
    )
    xs = x.reshape(NCORES, BL, D)
    return [{"x": xs[i], "labels": labels_i32[i]} for i in range(NCORES)]


def finalize(results):
    total = 0.0
    for r in results:
        vals = np.asarray(r["out"], dtype=np.float64)
        total += float(np.clip(vals, 1e-12, 1e12).sum())
    loss = (total + B * (C - 1) * 1e-12) / B
    return np.array(loss, dtype=np.float32)


def kernel(x, labels, centers):
    nc = _get_nc(centers)
    in_maps = make_in_maps(x, labels)
    res = run_bass_kernel_spmd(nc, in_maps, core_ids=list(range(NCORES)))
    return finalize(res.results)
